# revision 21
# baseline (speedup 1.0000x reference)
"""GPTQ int4 dequant + matmul kernel for Trainium2, column-parallel over 8 cores.

Computes out = x @ dequant(qweight, qzeros, scales) + bias where
  qweight: [OC//8, IC_total] int32 (nibbles packed along OC rows)
  qzeros:  [G, IC_total//8]  int32 (nibbles packed along IC cols)
  scales:  [G, IC_total]     float32
  x:       [N, OC]           float32
  bias:    [IC_total]        float32
Sharding: IC (out_features) split across 8 cores; x replicated.

v3 schedule (v1 baseline ~825us: W-prep ran serially up front, PE idle
~180us).  Changes:
 - W-prep stays on DVE ([j-part, k-free] unpack + fused dequant with
   per-partition zp/s scalars, then xbar transpose to k-major chunks) but in
   (j-tile, k-half) units so chunk 0 of W is ready ~45us in.
 - PSUM accumulators are per-chunk ([128, cw<=512] = 1 bank), so matmuls
   stream chunk-by-chunk as W chunks appear: early token tiles run
   (chunk, nt)-blocked, later ones kt-major for stationary reuse.
 - PE-transpose drains go to the scalar (ACT) engine, off the DVE.
 - bias is folded into the PSUM drain (DVE scalar_tensor_tensor add with a
   DMA-broadcast bias tile); no bias matmuls.
 - x tiles (gpsimd cast-DMA fp32->bf16 + xbar transpose) prefetch from t=0.
"""

import sys

if "/opt/trn_rl_repo" not in sys.path:
    sys.path.insert(0, "/opt/trn_rl_repo")

from contextlib import ExitStack

import numpy as np
import ml_dtypes

from concourse import bacc, bass, mybir, tile

P = 128
PACK = 8
FP32_BIAS_BITS = 0x4B000000  # fp32 bit pattern of 2**23
FP32_BIAS = float(2**23)

f32 = mybir.dt.float32
bf16 = mybir.dt.bfloat16
i32 = mybir.dt.int32
Alu = mybir.AluOpType

# Full problem dims (hardcoded per harness contract)
N_FULL = 4096
K_FULL = 4096  # OC / in_features (contraction)
IC_TOTAL = 11008
G_FULL = 32
N_CORES = 8
IC_SHARD = IC_TOTAL // N_CORES  # 1376

EARLY_NTS = 5  # token tiles processed (chunk, nt)-blocked during W-prep


def _jtiles(ic):
    """IC j-tiles of <=128, last may be ragged (must stay %16 for xbar)."""
    tiles = []
    off = 0
    while off < ic:
        w = min(P, ic - off)
        assert w % 16 == 0, f"ragged j-tile {w} not multiple of 16"
        tiles.append((off, w))
        off += w
    return tiles


def _chunks(ic):
    """Greedy grouping of j-tiles into psum chunks of <=512 fp32."""
    chunks = []
    start = 0
    for off, w in _jtiles(ic):
        if off + w - start > 512:
            chunks.append((start, off - start))
            start = off
    chunks.append((start, ic - start))
    return chunks


def build(nc, n=N_FULL, k=K_FULL, ic=IC_SHARD, g=G_FULL):
    """Emit the per-core program. All cores run the same program (SPMD)."""
    assert k % P == 0 and n % P == 0 and k // g == P
    KT = k // P  # contraction tiles (each == one quant group)
    NT = n // P  # token tiles
    jts = _jtiles(ic)
    NJ = len(jts)
    chunks = _chunks(ic)
    NC = len(chunks)
    jt_chunk = []
    chunk_jts = [[] for _ in chunks]
    for ji, (off, w) in enumerate(jts):
        for ci, (c0, cw) in enumerate(chunks):
            if c0 <= off < c0 + cw:
                jt_chunk.append((ci, off - c0))
                chunk_jts[ci].append(ji)
                break

    KH = KT // 2  # k-half in tiles (16)

    q_d = nc.dram_tensor("qweight", [k // PACK, ic], i32, kind="ExternalInput")
    qz_d = nc.dram_tensor("qzeros", [g, ic // PACK], i32, kind="ExternalInput")
    s_d = nc.dram_tensor("scales", [g, ic], f32, kind="ExternalInput")
    x_d = nc.dram_tensor("x", [n, k], f32, kind="ExternalInput")
    b_d = nc.dram_tensor("bias", [ic], f32, kind="ExternalInput")
    id128_d = nc.dram_tensor("id128_f32", [P, P], f32, kind="ExternalInput")
    idg_f_d = nc.dram_tensor("idg_f32", [g, g], f32, kind="ExternalInput")
    out_d = nc.dram_tensor("out", [n, ic], f32, kind="ExternalOutput")

    with tile.TileContext(nc) as tc, ExitStack() as ctx:
        const = ctx.enter_context(tc.tile_pool(name="const", bufs=1))
        wpool = ctx.enter_context(tc.tile_pool(name="w", bufs=1))
        prep = ctx.enter_context(tc.tile_pool(name="prep", bufs=1))
        xbpool = ctx.enter_context(tc.tile_pool(name="xb", bufs=2))
        xtpool = ctx.enter_context(tc.tile_pool(name="xt", bufs=EARLY_NTS + 1))
        opool = ctx.enter_context(tc.tile_pool(name="o", bufs=2))
        psum = ctx.enter_context(tc.tile_pool(name="psum", bufs=2, space="PSUM"))
        psum_t = ctx.enter_context(tc.tile_pool(name="psum_t", bufs=2, space="PSUM"))

        # ---- constants / broadcast rows
        id128 = const.tile([P, P], f32)
        nc.sync.dma_start(out=id128[:], in_=id128_d[:])
        idg_f = const.tile([g, g], f32)
        nc.sync.dma_start(out=idg_f[:], in_=idg_f_d[:])
        bias_bc = const.tile([P, ic], f32)
        nc.sync.dma_start(out=bias_bc[:], in_=b_d[None, :].broadcast_to([P, ic]))
        ones = const.tile([1, P], bf16)
        nc.vector.memset(ones[:], 1.0)
        bias_row = const.tile([1, ic], bf16)
        nc.gpsimd.dma_start(out=bias_row[:], in_=b_d[None, :])  # cast f32->bf16

        # ---- x tiles: gpsimd cast-DMA fp32->bf16, xbar transpose to [k, n]
        def load_x(nt):
            xb = xbpool.tile([P, k], bf16, name="xb")
            nc.gpsimd.dma_start(out=xb[:], in_=x_d[nt * P : (nt + 1) * P, :])
            xT = xtpool.tile([P, KT, P], bf16, name="xT")
            nc.scalar.dma_start_transpose(out=xT[:], in_=xb[:])
            return xT

        xts = {}
        for nt in range(EARLY_NTS):  # prefetch early tiles immediately
            xts[nt] = load_x(nt)

        # ---- zp unpack: qzeros [g, ic//8] -> zp_or [g, ic] (bits = fp32 2^23+zp)
        qz_sb = const.tile([g, ic // PACK], i32)
        nc.sync.dma_start(out=qz_sb[:], in_=qz_d[:])
        zp_or = const.tile([g, ic], i32)
        for r in range(PACK):
            nc.vector.tensor_scalar(
                out=zp_or[:, r::PACK],
                in0=qz_sb[:],
                scalar1=4 * r,
                scalar2=15,
                op0=Alu.logical_shift_right,
                op1=Alu.bitwise_and,
            )
        nc.vector.tensor_scalar(
            out=zp_or[:], in0=zp_or[:], scalar1=FP32_BIAS_BITS, scalar2=None,
            op0=Alu.bitwise_or,
        )
        s_sb = const.tile([g, ic], f32)
        nc.sync.dma_start(out=s_sb[:], in_=s_d[:])

        # ---- transpose zp_or and scales to [IC-part, g] layout (PE + ACT)
        zpT = const.tile([P, NJ, g], f32)  # bits are fp32 2^23+zp already
        sT = const.tile([P, NJ, g], f32)
        for ji, (off, w) in enumerate(jts):
            pz = psum_t.tile([P, P], f32, name="pst_f")
            nc.tensor.transpose(
                pz[:w, :g], zp_or.bitcast(f32)[:, off : off + w], idg_f[:]
            )
            nc.scalar.copy(out=zpT[:w, ji, :], in_=pz[:w, :g])
            ps_ = psum_t.tile([P, P], f32, name="pst_f")
            nc.tensor.transpose(ps_[:w, :g], s_sb[:, off : off + w], idg_f[:])
            nc.scalar.copy(out=sT[:w, ji, :], in_=ps_[:w, :g])

        # plain zp values (exact): zpT holds 2^23+zp as f32
        zpP = const.tile([P, NJ, g], f32)
        nc.vector.tensor_scalar(
            out=zpP[:], in0=zpT[:], scalar1=FP32_BIAS, scalar2=None,
            op0=Alu.subtract,
        )

        # ---- W chunks in [OC-part, KT, chunk-width] bf16
        wtiles = [wpool.tile([P, KT, cw], bf16, name=f"Wc{ci}")
                  for ci, (c0, cw) in enumerate(chunks)]

        RH = (k // PACK) // 2  # packed rows per k-half (256)

        def prep_unit(ji, h):
            """Dequantize j-tile ji, k-half h (DVE + ACT psum drains)."""
            off, w = jts[ji]
            qw4 = prep.tile([P, 2, P], i32, name="qw4", bufs=2)
            for rt in range(2):
                r0 = h * RH + rt * P
                nc.sync.dma_start(
                    out=qw4[:, rt, :w], in_=q_d[r0 : r0 + P, off : off + w]
                )
            qwT = prep.tile([P, 2 * P], i32, name="qwT")
            for rt in range(2):
                pq = psum_t.tile([P, P], f32, name="pst_f")
                nc.tensor.transpose(
                    pq[:w, :], qw4.bitcast(f32)[:, rt, :w], id128[:]
                )
                # must be DVE: ACT copies are not bit-exact for raw int bits
                nc.vector.tensor_copy(
                    qwT.bitcast(f32)[:w, rt * P : (rt + 1) * P], pq[:w, :]
                )
            # unpack nibbles: nib[j, 8r+kk] = (qwT[j, r] >> 4kk) & 15
            # (bitwise ts ops cannot cast: int32 out)
            nib = prep.tile([P, k // 2], i32, name="nib")
            for kk in range(PACK):
                nc.vector.tensor_scalar(
                    out=nib[:w, kk::PACK],
                    in0=qwT[:w, :],
                    scalar1=4 * kk,
                    scalar2=15,
                    op0=Alu.logical_shift_right,
                    op1=Alu.bitwise_and,
                )
            # dequant: WT = (nib - zp) * s -> bf16; arith ts converts the int32
            # nibbles numerically (f32-internal, exact), one rounding to bf16
            wt = prep.tile([P, k // 2], bf16, name="wt", bufs=2)
            for gi in range(g // 2):
                gg = h * (g // 2) + gi
                nc.vector.tensor_scalar(
                    out=wt[:w, gi * P : (gi + 1) * P],
                    in0=nib[:w, gi * P : (gi + 1) * P],
                    scalar1=zpP[:w, ji, gg : gg + 1],
                    scalar2=sT[:w, ji, gg : gg + 1],
                    op0=Alu.subtract,
                    op1=Alu.mult,
                )
            ci, coff = jt_chunk[ji]
            nc.sync.dma_start_transpose(
                out=wtiles[ci][:, h * KH : (h + 1) * KH, coff : coff + w],
                in_=wt[:w, :],
            )



        # ---- matmul passes: per-chunk PSUM accumulation (1 bank each)
        # Early passes must not touch the DVE: its strict FIFO is full of prep
        # work, so a DVE drain would stall the PSUM ring.  Bias comes from a
        # K=1 ones-row matmul; the drain is an ACT copy.
        def mm_early_A(nt, ci, xT):
            c0, cw = chunks[ci]
            ps = psum.tile([P, 512], f32, name=f"ps{ci}")
            for kt in range(KH):
                nc.tensor.matmul(
                    ps[:, :cw], lhsT=xT[:, kt, :], rhs=wtiles[ci][:, kt, :],
                    start=(kt == 0), stop=False,
                )
            return ps

        def mm_early_B(nt, ci, xT, ps):
            c0, cw = chunks[ci]
            for kt in range(KH, KT):
                nc.tensor.matmul(
                    ps[:, :cw], lhsT=xT[:, kt, :], rhs=wtiles[ci][:, kt, :],
                    start=False, stop=False,
                )
            nc.tensor.matmul(
                ps[:, :cw], lhsT=ones[:, :], rhs=bias_row[:, c0 : c0 + cw],
                start=False, stop=True,
            )
            out_sb = opool.tile([P, 512], f32, name=f"ob{ci}")
            nc.scalar.copy(out=out_sb[:, :cw], in_=ps[:, :cw])
            nc.sync.dma_start(
                out=out_d[nt * P : (nt + 1) * P, c0 : c0 + cw],
                in_=out_sb[:, :cw],
            )

        # prep per chunk: all k-half-0 units first (enables kt 0..15 matmuls
        # for the whole chunk), then the k-half-1 units
        for ci in range(NC):
            for h in (0, 1):
                for ji in chunk_jts[ci]:
                    prep_unit(ji, h)

        # early block: per chunk, pairs of token tiles run their kt0-15 bursts
        # as soon as the chunk's k-half-0 is ready, kt16-31 when half-1 lands.
        for ci in range(NC):
            nt = 0
            while nt < EARLY_NTS:
                pair = [n for n in (nt, nt + 1) if n < EARLY_NTS]
                pss = {n: mm_early_A(n, ci, xts[n]) for n in pair}
                for n in pair:
                    mm_early_B(n, ci, xts[n], pss[n])
                nt += 2

        # steady state: kt-major per token tile (better stationary reuse)
        for nt in range(EARLY_NTS, NT):
            xT = load_x(nt)
            pss = [psum.tile([P, 512], f32, name=f"ps{ci}") for ci in range(NC)]
            for kt in range(KT):
                for ci, (c0, cw) in enumerate(chunks):
                    nc.tensor.matmul(
                        pss[ci][:, :cw],
                        lhsT=xT[:, kt, :],
                        rhs=wtiles[ci][:, kt, :],
                        start=(kt == 0),
                        stop=(kt == KT - 1),
                    )
            for ci, (c0, cw) in enumerate(chunks):
                out_sb = opool.tile([P, 512], f32, name=f"ob{ci}")
                nc.vector.scalar_tensor_tensor(
                    out=out_sb[:, :cw], in0=pss[ci][:, :cw], scalar=0.0,
                    in1=bias_bc[:, c0 : c0 + cw], op0=Alu.add, op1=Alu.add,
                )
                nc.sync.dma_start(
                    out=out_d[nt * P : (nt + 1) * P, c0 : c0 + cw],
                    in_=out_sb[:, :cw],
                )
    return nc


def make_const_inputs(g=G_FULL):
    return {
        "id128_f32": np.eye(P, dtype=np.float32),
        "idg_f32": np.eye(g, dtype=np.float32),
    }


def kernel(input, qweight, qzeros, scales, bias):
    """Full-problem entry point: shard, run on 8 cores, gather."""
    from concourse.bass_utils import run_bass_kernel_spmd

    nc = bacc.Bacc("TRN2", target_bir_lowering=False, debug=False)
    build(nc)
    nc.compile()

    consts = make_const_inputs()
    x = np.ascontiguousarray(input, dtype=np.float32)
    in_maps = []
    for c in range(N_CORES):
        j0, j1 = c * IC_SHARD, (c + 1) * IC_SHARD
        in_maps.append(
            {
                "qweight": np.ascontiguousarray(qweight[:, j0:j1]),
                "qzeros": np.ascontiguousarray(
                    qzeros[:, c * (IC_SHARD // PACK) : (c + 1) * (IC_SHARD // PACK)]
                ),
                "scales": np.ascontiguousarray(scales[:, j0:j1]),
                "x": x,
                "bias": np.ascontiguousarray(bias[j0:j1]),
                **consts,
            }
        )
    res = run_bass_kernel_spmd(nc, in_maps, list(range(N_CORES)))
    outs = [np.asarray(res.results[c]["out"], dtype=np.float32) for c in range(N_CORES)]
    return np.concatenate(outs, axis=1)


# revision 22
# speedup vs baseline: 1.1330x; 1.1330x over previous
"""GPTQ int4 dequant + matmul kernel for Trainium2, column-parallel over 8 cores.

Computes out = x @ dequant(qweight, qzeros, scales) + bias where
  qweight: [OC//8, IC_total] int32 (nibbles packed along OC rows)
  qzeros:  [G, IC_total//8]  int32 (nibbles packed along IC cols)
  scales:  [G, IC_total]     float32
  x:       [N, OC]           float32
  bias:    [IC_total]        float32
Sharding: IC (out_features) split across 8 cores; x replicated.

v3 schedule (v1 baseline ~825us: W-prep ran serially up front, PE idle
~180us).  Changes:
 - W-prep stays on DVE ([j-part, k-free] unpack + fused dequant with
   per-partition zp/s scalars, then xbar transpose to k-major chunks) but in
   (j-tile, k-half) units so chunk 0 of W is ready ~45us in.
 - PSUM accumulators are per-chunk ([128, cw<=512] = 1 bank), so matmuls
   stream chunk-by-chunk as W chunks appear: early token tiles run
   (chunk, nt)-blocked, later ones kt-major for stationary reuse.
 - PE-transpose drains go to the scalar (ACT) engine, off the DVE.
 - bias is folded into the PSUM drain (DVE scalar_tensor_tensor add with a
   DMA-broadcast bias tile); no bias matmuls.
 - x tiles (gpsimd cast-DMA fp32->bf16 + xbar transpose) prefetch from t=0.
"""

import sys

if "/opt/trn_rl_repo" not in sys.path:
    sys.path.insert(0, "/opt/trn_rl_repo")

from contextlib import ExitStack

import numpy as np
import ml_dtypes

from concourse import bacc, bass, mybir, tile

P = 128
PACK = 8
FP32_BIAS_BITS = 0x4B000000  # fp32 bit pattern of 2**23
FP32_BIAS = float(2**23)

f32 = mybir.dt.float32
bf16 = mybir.dt.bfloat16
i32 = mybir.dt.int32
Alu = mybir.AluOpType

# Full problem dims (hardcoded per harness contract)
N_FULL = 4096
K_FULL = 4096  # OC / in_features (contraction)
IC_TOTAL = 11008
G_FULL = 32
N_CORES = 8
IC_SHARD = IC_TOTAL // N_CORES  # 1376

EARLY_NTS = 5  # token tiles processed (chunk, nt)-blocked during W-prep


def _jtiles(ic):
    """IC j-tiles of <=128, last may be ragged (must stay %16 for xbar)."""
    tiles = []
    off = 0
    while off < ic:
        w = min(P, ic - off)
        assert w % 16 == 0, f"ragged j-tile {w} not multiple of 16"
        tiles.append((off, w))
        off += w
    return tiles


def _chunks(ic):
    """Greedy grouping of j-tiles into psum chunks of <=512 fp32."""
    chunks = []
    start = 0
    for off, w in _jtiles(ic):
        if off + w - start > 512:
            chunks.append((start, off - start))
            start = off
    chunks.append((start, ic - start))
    return chunks


def build(nc, n=N_FULL, k=K_FULL, ic=IC_SHARD, g=G_FULL):
    """Emit the per-core program. All cores run the same program (SPMD)."""
    assert k % P == 0 and n % P == 0 and k // g == P
    KT = k // P  # contraction tiles (each == one quant group)
    NT = n // P  # token tiles
    jts = _jtiles(ic)
    NJ = len(jts)
    chunks = _chunks(ic)
    NC = len(chunks)
    jt_chunk = []
    chunk_jts = [[] for _ in chunks]
    for ji, (off, w) in enumerate(jts):
        for ci, (c0, cw) in enumerate(chunks):
            if c0 <= off < c0 + cw:
                jt_chunk.append((ci, off - c0))
                chunk_jts[ci].append(ji)
                break

    KH = KT // 2  # k-half in tiles (16)

    q_d = nc.dram_tensor("qweight", [k // PACK, ic], i32, kind="ExternalInput")
    qz_d = nc.dram_tensor("qzeros", [g, ic // PACK], i32, kind="ExternalInput")
    s_d = nc.dram_tensor("scales", [g, ic], f32, kind="ExternalInput")
    x_d = nc.dram_tensor("x", [n, k], f32, kind="ExternalInput")
    b_d = nc.dram_tensor("bias", [ic], f32, kind="ExternalInput")
    id128_d = nc.dram_tensor("id128_f32", [P, P], f32, kind="ExternalInput")
    idg_f_d = nc.dram_tensor("idg_f32", [g, g], f32, kind="ExternalInput")
    out_d = nc.dram_tensor("out", [n, ic], f32, kind="ExternalOutput")

    with tile.TileContext(nc) as tc, ExitStack() as ctx:
        const = ctx.enter_context(tc.tile_pool(name="const", bufs=1))
        wpool = ctx.enter_context(tc.tile_pool(name="w", bufs=1))
        prep = ctx.enter_context(tc.tile_pool(name="prep", bufs=1))
        xbpool = ctx.enter_context(tc.tile_pool(name="xb", bufs=2))
        xtpool = ctx.enter_context(tc.tile_pool(name="xt", bufs=EARLY_NTS + 1))
        opool = ctx.enter_context(tc.tile_pool(name="o", bufs=2))
        psum = ctx.enter_context(tc.tile_pool(name="psum", bufs=2, space="PSUM"))
        psum_t = ctx.enter_context(tc.tile_pool(name="psum_t", bufs=2, space="PSUM"))

        # ---- constants / broadcast rows
        id128 = const.tile([P, P], f32)
        nc.sync.dma_start(out=id128[:], in_=id128_d[:])
        idg_f = const.tile([g, g], f32)
        nc.sync.dma_start(out=idg_f[:], in_=idg_f_d[:])
        bias_bc = const.tile([P, ic], f32)
        nc.sync.dma_start(out=bias_bc[:], in_=b_d[None, :].broadcast_to([P, ic]))
        ones = const.tile([1, P], bf16)
        nc.vector.memset(ones[:], 1.0)
        bias_row = const.tile([1, ic], bf16)
        nc.gpsimd.dma_start(out=bias_row[:], in_=b_d[None, :])  # cast f32->bf16

        # ---- x tiles: gpsimd cast-DMA fp32->bf16, xbar transpose to [k, n]
        def load_x(nt):
            xb = xbpool.tile([P, k], bf16, name="xb")
            nc.gpsimd.dma_start(out=xb[:], in_=x_d[nt * P : (nt + 1) * P, :])
            xT = xtpool.tile([P, KT, P], bf16, name="xT")
            nc.scalar.dma_start_transpose(out=xT[:], in_=xb[:])
            return xT

        xts = {}
        for nt in range(EARLY_NTS):  # prefetch early tiles immediately
            xts[nt] = load_x(nt)

        # ---- zp unpack: qzeros [g, ic//8] -> zp_or [g, ic] (bits = fp32 2^23+zp)
        qz_sb = const.tile([g, ic // PACK], i32)
        nc.sync.dma_start(out=qz_sb[:], in_=qz_d[:])
        zp_or = const.tile([g, ic], i32)
        for r in range(PACK):
            nc.vector.tensor_scalar(
                out=zp_or[:, r::PACK],
                in0=qz_sb[:],
                scalar1=4 * r,
                scalar2=15,
                op0=Alu.logical_shift_right,
                op1=Alu.bitwise_and,
            )
        nc.vector.tensor_scalar(
            out=zp_or[:], in0=zp_or[:], scalar1=FP32_BIAS_BITS, scalar2=None,
            op0=Alu.bitwise_or,
        )
        s_sb = const.tile([g, ic], f32)
        nc.sync.dma_start(out=s_sb[:], in_=s_d[:])

        # ---- transpose zp_or and scales to [IC-part, g] layout (PE + ACT)
        zpT = const.tile([P, NJ, g], f32)  # bits are fp32 2^23+zp already
        sT = const.tile([P, NJ, g], f32)
        for ji, (off, w) in enumerate(jts):
            pz = psum_t.tile([P, P], f32, name="pst_f")
            nc.tensor.transpose(
                pz[:w, :g], zp_or.bitcast(f32)[:, off : off + w], idg_f[:]
            )
            nc.scalar.copy(out=zpT[:w, ji, :], in_=pz[:w, :g])
            ps_ = psum_t.tile([P, P], f32, name="pst_f")
            nc.tensor.transpose(ps_[:w, :g], s_sb[:, off : off + w], idg_f[:])
            nc.scalar.copy(out=sT[:w, ji, :], in_=ps_[:w, :g])

        # plain zp values (exact): zpT holds 2^23+zp as f32
        zpP = const.tile([P, NJ, g], f32)
        nc.vector.tensor_scalar(
            out=zpP[:], in0=zpT[:], scalar1=FP32_BIAS, scalar2=None,
            op0=Alu.subtract,
        )

        # ---- W chunks in [OC-part, KT, chunk-width] bf16
        wtiles = [wpool.tile([P, KT, cw], bf16, name=f"Wc{ci}")
                  for ci, (c0, cw) in enumerate(chunks)]

        RH = (k // PACK) // 2  # packed rows per k-half (256)

        def unit_load(ji, h):
            # qweight loads ride the ACT hwdge queue so the sync queue (which
            # carries the dequant-gated xbar transposes) never blocks them
            off, w = jts[ji]
            qw4 = prep.tile([P, 2, P], i32, name="qw4", bufs=2)
            for rt in range(2):
                r0 = h * RH + rt * P
                nc.scalar.dma_start(
                    out=qw4[:, rt, :w], in_=q_d[r0 : r0 + P, off : off + w]
                )
            return qw4

        def prep_unit(ji, h, qw4):
            """Dequantize j-tile ji, k-half h."""
            off, w = jts[ji]
            qwT = prep.tile([P, 2 * P], i32, name="qwT")
            for rt in range(2):
                pq = psum_t.tile([P, P], f32, name="pst_f")
                nc.tensor.transpose(
                    pq[:w, :], qw4.bitcast(f32)[:, rt, :w], id128[:]
                )
                # must be DVE: ACT copies are not bit-exact for raw int bits
                nc.vector.tensor_copy(
                    qwT.bitcast(f32)[:w, rt * P : (rt + 1) * P], pq[:w, :]
                )
            # unpack nibbles: nib[j, 8r+kk] = (qwT[j, r] >> 4kk) & 15
            # (bitwise ts ops cannot cast: int32 out)
            nib = prep.tile([P, k // 2], i32, name="nib")
            for kk in range(PACK):
                nc.vector.tensor_scalar(
                    out=nib[:w, kk::PACK],
                    in0=qwT[:w, :],
                    scalar1=4 * kk,
                    scalar2=15,
                    op0=Alu.logical_shift_right,
                    op1=Alu.bitwise_and,
                )
            # dequant: WT = (nib - zp) * s -> bf16; arith ts converts the int32
            # nibbles numerically (f32-internal, exact), one rounding to bf16
            wt = prep.tile([P, k // 2], bf16, name="wt", bufs=2)
            for gi in range(g // 2):
                gg = h * (g // 2) + gi
                nc.vector.tensor_scalar(
                    out=wt[:w, gi * P : (gi + 1) * P],
                    in0=nib[:w, gi * P : (gi + 1) * P],
                    scalar1=zpP[:w, ji, gg : gg + 1],
                    scalar2=sT[:w, ji, gg : gg + 1],
                    op0=Alu.subtract,
                    op1=Alu.mult,
                )
            ci, coff = jt_chunk[ji]
            nc.sync.dma_start_transpose(
                out=wtiles[ci][:, h * KH : (h + 1) * KH, coff : coff + w],
                in_=wt[:w, :],
            )



        # ---- matmul passes: per-chunk PSUM accumulation (1 bank each)
        # Early passes must not touch the DVE: its strict FIFO is full of prep
        # work, so a DVE drain would stall the PSUM ring.  Bias comes from a
        # K=1 ones-row matmul; the drain is an ACT copy.
        def mm_early_A(nt, ci, xT):
            c0, cw = chunks[ci]
            ps = psum.tile([P, 512], f32, name=f"ps{ci}")
            for kt in range(KH):
                nc.tensor.matmul(
                    ps[:, :cw], lhsT=xT[:, kt, :], rhs=wtiles[ci][:, kt, :],
                    start=(kt == 0), stop=False,
                )
            return ps

        def mm_early_B(nt, ci, xT, ps):
            c0, cw = chunks[ci]
            for kt in range(KH, KT):
                nc.tensor.matmul(
                    ps[:, :cw], lhsT=xT[:, kt, :], rhs=wtiles[ci][:, kt, :],
                    start=False, stop=False,
                )
            nc.tensor.matmul(
                ps[:, :cw], lhsT=ones[:, :], rhs=bias_row[:, c0 : c0 + cw],
                start=False, stop=True,
            )
            out_sb = opool.tile([P, 512], f32, name=f"ob{ci}")
            nc.scalar.copy(out=out_sb[:, :cw], in_=ps[:, :cw])
            nc.sync.dma_start(
                out=out_d[nt * P : (nt + 1) * P, c0 : c0 + cw],
                in_=out_sb[:, :cw],
            )

        # software-pipelined prep + early passes: chunk c's computes follow its
        # loads; chunk c+1's loads are emitted before chunk c's passes so the
        # ACT queue (loads + drains) and sync queue (xbar transposes) never
        # stall each other; h0 units first so kt 0..15 matmuls start early.
        chunk_units = [
            [(ji, h) for h in (0, 1) for ji in chunk_jts[ci]] for ci in range(NC)
        ]
        loaded = {u: unit_load(*u) for u in chunk_units[0]}

        def emit_passes(ci):
            nt = 0
            while nt < EARLY_NTS:
                pair = [n for n in (nt, nt + 1) if n < EARLY_NTS]
                pss = {n: mm_early_A(n, ci, xts[n]) for n in pair}
                for n in pair:
                    mm_early_B(n, ci, xts[n], pss[n])
                nt += 2

        for ci in range(NC):
            for u in chunk_units[ci]:
                prep_unit(*u, loaded[u])
            if ci + 1 < NC:
                for u in chunk_units[ci + 1]:
                    loaded[u] = unit_load(*u)
            emit_passes(ci)

        # steady state: kt-major per token tile (better stationary reuse)
        for nt in range(EARLY_NTS, NT):
            xT = load_x(nt)
            pss = [psum.tile([P, 512], f32, name=f"ps{ci}") for ci in range(NC)]
            for kt in range(KT):
                for ci, (c0, cw) in enumerate(chunks):
                    nc.tensor.matmul(
                        pss[ci][:, :cw],
                        lhsT=xT[:, kt, :],
                        rhs=wtiles[ci][:, kt, :],
                        start=(kt == 0),
                        stop=(kt == KT - 1),
                    )
            for ci, (c0, cw) in enumerate(chunks):
                out_sb = opool.tile([P, 512], f32, name=f"ob{ci}")
                nc.vector.scalar_tensor_tensor(
                    out=out_sb[:, :cw], in0=pss[ci][:, :cw], scalar=0.0,
                    in1=bias_bc[:, c0 : c0 + cw], op0=Alu.add, op1=Alu.add,
                )
                nc.sync.dma_start(
                    out=out_d[nt * P : (nt + 1) * P, c0 : c0 + cw],
                    in_=out_sb[:, :cw],
                )
    return nc


def make_const_inputs(g=G_FULL):
    return {
        "id128_f32": np.eye(P, dtype=np.float32),
        "idg_f32": np.eye(g, dtype=np.float32),
    }


def kernel(input, qweight, qzeros, scales, bias):
    """Full-problem entry point: shard, run on 8 cores, gather."""
    from concourse.bass_utils import run_bass_kernel_spmd

    nc = bacc.Bacc("TRN2", target_bir_lowering=False, debug=False)
    build(nc)
    nc.compile()

    consts = make_const_inputs()
    x = np.ascontiguousarray(input, dtype=np.float32)
    in_maps = []
    for c in range(N_CORES):
        j0, j1 = c * IC_SHARD, (c + 1) * IC_SHARD
        in_maps.append(
            {
                "qweight": np.ascontiguousarray(qweight[:, j0:j1]),
                "qzeros": np.ascontiguousarray(
                    qzeros[:, c * (IC_SHARD // PACK) : (c + 1) * (IC_SHARD // PACK)]
                ),
                "scales": np.ascontiguousarray(scales[:, j0:j1]),
                "x": x,
                "bias": np.ascontiguousarray(bias[j0:j1]),
                **consts,
            }
        )
    res = run_bass_kernel_spmd(nc, in_maps, list(range(N_CORES)))
    outs = [np.asarray(res.results[c]["out"], dtype=np.float32) for c in range(N_CORES)]
    return np.concatenate(outs, axis=1)


# revision 24
# speedup vs baseline: 1.1485x; 1.0137x over previous
"""GPTQ int4 dequant + matmul kernel for Trainium2, column-parallel over 8 cores.

Computes out = x @ dequant(qweight, qzeros, scales) + bias where
  qweight: [OC//8, IC_total] int32 (nibbles packed along OC rows)
  qzeros:  [G, IC_total//8]  int32 (nibbles packed along IC cols)
  scales:  [G, IC_total]     float32
  x:       [N, OC]           float32
  bias:    [IC_total]        float32
Sharding: IC (out_features) split across 8 cores; x replicated.

v3 schedule (v1 baseline ~825us: W-prep ran serially up front, PE idle
~180us).  Changes:
 - W-prep stays on DVE ([j-part, k-free] unpack + fused dequant with
   per-partition zp/s scalars, then xbar transpose to k-major chunks) but in
   (j-tile, k-half) units so chunk 0 of W is ready ~45us in.
 - PSUM accumulators are per-chunk ([128, cw<=512] = 1 bank), so matmuls
   stream chunk-by-chunk as W chunks appear: early token tiles run
   (chunk, nt)-blocked, later ones kt-major for stationary reuse.
 - PE-transpose drains go to the scalar (ACT) engine, off the DVE.
 - bias is folded into the PSUM drain (DVE scalar_tensor_tensor add with a
   DMA-broadcast bias tile); no bias matmuls.
 - x tiles (gpsimd cast-DMA fp32->bf16 + xbar transpose) prefetch from t=0.
"""

import sys

if "/opt/trn_rl_repo" not in sys.path:
    sys.path.insert(0, "/opt/trn_rl_repo")

from contextlib import ExitStack

import numpy as np
import ml_dtypes

from concourse import bacc, bass, mybir, tile

P = 128
PACK = 8
FP32_BIAS_BITS = 0x4B000000  # fp32 bit pattern of 2**23
FP32_BIAS = float(2**23)

f32 = mybir.dt.float32
bf16 = mybir.dt.bfloat16
i32 = mybir.dt.int32
Alu = mybir.AluOpType

# Full problem dims (hardcoded per harness contract)
N_FULL = 4096
K_FULL = 4096  # OC / in_features (contraction)
IC_TOTAL = 11008
G_FULL = 32
N_CORES = 8
IC_SHARD = IC_TOTAL // N_CORES  # 1376

EARLY_NTS = 5  # token tiles processed (chunk, nt)-blocked during W-prep


def _jtiles(ic):
    """IC j-tiles of <=128, last may be ragged (must stay %16 for xbar)."""
    tiles = []
    off = 0
    while off < ic:
        w = min(P, ic - off)
        assert w % 16 == 0, f"ragged j-tile {w} not multiple of 16"
        tiles.append((off, w))
        off += w
    return tiles


def _chunks(ic):
    """Greedy grouping of j-tiles into psum chunks of <=512 fp32."""
    chunks = []
    start = 0
    for off, w in _jtiles(ic):
        if off + w - start > 512:
            chunks.append((start, off - start))
            start = off
    chunks.append((start, ic - start))
    return chunks


def build(nc, n=N_FULL, k=K_FULL, ic=IC_SHARD, g=G_FULL):
    """Emit the per-core program. All cores run the same program (SPMD)."""
    assert k % P == 0 and n % P == 0 and k // g == P
    KT = k // P  # contraction tiles (each == one quant group)
    NT = n // P  # token tiles
    jts = _jtiles(ic)
    NJ = len(jts)
    chunks = _chunks(ic)
    NC = len(chunks)
    jt_chunk = []
    chunk_jts = [[] for _ in chunks]
    for ji, (off, w) in enumerate(jts):
        for ci, (c0, cw) in enumerate(chunks):
            if c0 <= off < c0 + cw:
                jt_chunk.append((ci, off - c0))
                chunk_jts[ci].append(ji)
                break

    KH = KT // 2  # k-half in tiles (16)

    q_d = nc.dram_tensor("qweight", [k // PACK, ic], i32, kind="ExternalInput")
    qz_d = nc.dram_tensor("qzeros", [g, ic // PACK], i32, kind="ExternalInput")
    s_d = nc.dram_tensor("scales", [g, ic], f32, kind="ExternalInput")
    x_d = nc.dram_tensor("x", [n, k], f32, kind="ExternalInput")
    b_d = nc.dram_tensor("bias", [ic], f32, kind="ExternalInput")
    id128_d = nc.dram_tensor("id128_f32", [P, P], f32, kind="ExternalInput")
    idg_f_d = nc.dram_tensor("idg_f32", [g, g], f32, kind="ExternalInput")
    out_d = nc.dram_tensor("out", [n, ic], f32, kind="ExternalOutput")

    with tile.TileContext(nc) as tc, ExitStack() as ctx:
        const = ctx.enter_context(tc.tile_pool(name="const", bufs=1))
        wpool = ctx.enter_context(tc.tile_pool(name="w", bufs=1))
        prep = ctx.enter_context(tc.tile_pool(name="prep", bufs=1))
        xbpool = ctx.enter_context(tc.tile_pool(name="xb", bufs=2))
        xtpool = ctx.enter_context(tc.tile_pool(name="xt", bufs=EARLY_NTS + 1))
        opool = ctx.enter_context(tc.tile_pool(name="o", bufs=2))
        psum = ctx.enter_context(tc.tile_pool(name="psum", bufs=2, space="PSUM"))
        psum_t = ctx.enter_context(tc.tile_pool(name="psum_t", bufs=2, space="PSUM"))

        # ---- constants / broadcast rows
        id128 = const.tile([P, P], f32)
        nc.sync.dma_start(out=id128[:], in_=id128_d[:])
        idg_f = const.tile([g, g], f32)
        nc.sync.dma_start(out=idg_f[:], in_=idg_f_d[:])
        bias_bc = const.tile([P, ic], f32)
        nc.sync.dma_start(out=bias_bc[:], in_=b_d[None, :].broadcast_to([P, ic]))
        ones = const.tile([1, P], bf16)
        nc.vector.memset(ones[:], 1.0)
        bias_row = const.tile([1, ic], bf16)
        nc.gpsimd.dma_start(out=bias_row[:], in_=b_d[None, :])  # cast f32->bf16

        # ---- x tiles: gpsimd cast-DMA fp32->bf16, xbar transpose to [k, n]
        def load_x(nt):
            xb = xbpool.tile([P, k], bf16, name="xb")
            nc.gpsimd.dma_start(out=xb[:], in_=x_d[nt * P : (nt + 1) * P, :])
            xT = xtpool.tile([P, KT, P], bf16, name="xT")
            nc.scalar.dma_start_transpose(out=xT[:], in_=xb[:])
            return xT

        xts = {}
        for nt in range(EARLY_NTS):  # prefetch early tiles immediately
            xts[nt] = load_x(nt)

        # ---- zp unpack: qzeros [g, ic//8] -> zp_or [g, ic] (bits = fp32 2^23+zp)
        qz_sb = const.tile([g, ic // PACK], i32)
        nc.sync.dma_start(out=qz_sb[:], in_=qz_d[:])
        zp_or = const.tile([g, ic], i32)
        for r in range(PACK):
            nc.vector.tensor_scalar(
                out=zp_or[:, r::PACK],
                in0=qz_sb[:],
                scalar1=4 * r,
                scalar2=15,
                op0=Alu.logical_shift_right,
                op1=Alu.bitwise_and,
            )
        nc.vector.tensor_scalar(
            out=zp_or[:], in0=zp_or[:], scalar1=FP32_BIAS_BITS, scalar2=None,
            op0=Alu.bitwise_or,
        )
        s_sb = const.tile([g, ic], f32)
        nc.sync.dma_start(out=s_sb[:], in_=s_d[:])

        # ---- transpose zp_or and scales to [IC-part, g] layout (PE + ACT)
        zpT = const.tile([P, NJ, g], f32)  # bits are fp32 2^23+zp already
        sT = const.tile([P, NJ, g], f32)
        zpP = const.tile([P, NJ, g], f32)
        for ji, (off, w) in enumerate(jts):
            pz = psum_t.tile([P, P], f32, name="pst_f")
            nc.tensor.transpose(
                pz[:w, :g], zp_or.bitcast(f32)[:, off : off + w], idg_f[:]
            )
            nc.scalar.copy(out=zpT[:w, ji, :], in_=pz[:w, :g])
            ps_ = psum_t.tile([P, P], f32, name="pst_f")
            nc.tensor.transpose(ps_[:w, :g], s_sb[:, off : off + w], idg_f[:])
            nc.scalar.copy(out=sT[:w, ji, :], in_=ps_[:w, :g])
            # plain zp (exact): per-ji so unit ji's dequant only waits its own
            nc.vector.tensor_scalar(
                out=zpP[:w, ji, :], in0=zpT[:w, ji, :], scalar1=FP32_BIAS,
                scalar2=None, op0=Alu.subtract,
            )

        # ---- W chunks in [OC-part, KT, chunk-width] bf16
        wtiles = [wpool.tile([P, KT, cw], bf16, name=f"Wc{ci}")
                  for ci, (c0, cw) in enumerate(chunks)]

        RH = (k // PACK) // 2  # packed rows per k-half (256)

        def prep_unit(ji, h):
            """Dequantize j-tile ji, k-half h (DVE + ACT psum drains)."""
            off, w = jts[ji]
            qw4 = prep.tile([P, 2, P], i32, name="qw4", bufs=3)
            for rt in range(2):
                r0 = h * RH + rt * P
                nc.sync.dma_start(
                    out=qw4[:, rt, :w], in_=q_d[r0 : r0 + P, off : off + w]
                )
            qwT = prep.tile([P, 2 * P], i32, name="qwT")
            for rt in range(2):
                pq = psum_t.tile([P, P], f32, name="pst_f")
                nc.tensor.transpose(
                    pq[:w, :], qw4.bitcast(f32)[:, rt, :w], id128[:]
                )
                # must be DVE: ACT copies are not bit-exact for raw int bits
                nc.vector.tensor_copy(
                    qwT.bitcast(f32)[:w, rt * P : (rt + 1) * P], pq[:w, :]
                )
            # unpack nibbles: nib[j, 8r+kk] = (qwT[j, r] >> 4kk) & 15
            # (bitwise ts ops cannot cast: int32 out)
            nib = prep.tile([P, k // 2], i32, name="nib")
            for kk in range(PACK):
                nc.vector.tensor_scalar(
                    out=nib[:w, kk::PACK],
                    in0=qwT[:w, :],
                    scalar1=4 * kk,
                    scalar2=15,
                    op0=Alu.logical_shift_right,
                    op1=Alu.bitwise_and,
                )
            # dequant: WT = (nib - zp) * s -> bf16; arith ts converts the int32
            # nibbles numerically (f32-internal, exact), one rounding to bf16
            wt = prep.tile([P, k // 2], bf16, name="wt", bufs=2)
            for gi in range(g // 2):
                gg = h * (g // 2) + gi
                nc.vector.tensor_scalar(
                    out=wt[:w, gi * P : (gi + 1) * P],
                    in0=nib[:w, gi * P : (gi + 1) * P],
                    scalar1=zpP[:w, ji, gg : gg + 1],
                    scalar2=sT[:w, ji, gg : gg + 1],
                    op0=Alu.subtract,
                    op1=Alu.mult,
                )
            ci, coff = jt_chunk[ji]
            nc.sync.dma_start_transpose(
                out=wtiles[ci][:, h * KH : (h + 1) * KH, coff : coff + w],
                in_=wt[:w, :],
            )



        # ---- matmul passes: per-chunk PSUM accumulation (1 bank each)
        # Early passes must not touch the DVE: its strict FIFO is full of prep
        # work, so a DVE drain would stall the PSUM ring.  Bias comes from a
        # K=1 ones-row matmul; the drain is an ACT copy.
        def mm_early_A(nt, ci, xT):
            c0, cw = chunks[ci]
            ps = psum.tile([P, 512], f32, name=f"ps{ci}")
            for kt in range(KH):
                nc.tensor.matmul(
                    ps[:, :cw], lhsT=xT[:, kt, :], rhs=wtiles[ci][:, kt, :],
                    start=(kt == 0), stop=False,
                )
            return ps

        def mm_early_B(nt, ci, xT, ps):
            c0, cw = chunks[ci]
            for kt in range(KH, KT):
                nc.tensor.matmul(
                    ps[:, :cw], lhsT=xT[:, kt, :], rhs=wtiles[ci][:, kt, :],
                    start=False, stop=False,
                )
            nc.tensor.matmul(
                ps[:, :cw], lhsT=ones[:, :], rhs=bias_row[:, c0 : c0 + cw],
                start=False, stop=True,
            )
            out_sb = opool.tile([P, 512], f32, name=f"ob{ci}")
            nc.scalar.copy(out=out_sb[:, :cw], in_=ps[:, :cw])
            nc.sync.dma_start(
                out=out_d[nt * P : (nt + 1) * P, c0 : c0 + cw],
                in_=out_sb[:, :cw],
            )

        # prep per chunk: all k-half-0 units first (enables kt 0..15 matmuls
        # for the whole chunk), then the k-half-1 units
        for ci in range(NC):
            for h in (0, 1):
                for ji in chunk_jts[ci]:
                    prep_unit(ji, h)

        # early block: per chunk, pairs of token tiles run their kt0-15 bursts
        # as soon as the chunk's k-half-0 is ready, kt16-31 when half-1 lands.
        for ci in range(NC):
            nt = 0
            while nt < EARLY_NTS:
                pair = [n for n in (nt, nt + 1) if n < EARLY_NTS]
                pss = {n: mm_early_A(n, ci, xts[n]) for n in pair}
                for n in pair:
                    mm_early_B(n, ci, xts[n], pss[n])
                nt += 2

        # steady state: kt-major per token tile (better stationary reuse)
        for nt in range(EARLY_NTS, NT):
            xT = load_x(nt)
            pss = [psum.tile([P, 512], f32, name=f"ps{ci}") for ci in range(NC)]
            for kt in range(KT):
                for ci, (c0, cw) in enumerate(chunks):
                    nc.tensor.matmul(
                        pss[ci][:, :cw],
                        lhsT=xT[:, kt, :],
                        rhs=wtiles[ci][:, kt, :],
                        start=(kt == 0),
                        stop=(kt == KT - 1),
                    )
            for ci, (c0, cw) in enumerate(chunks):
                out_sb = opool.tile([P, 512], f32, name=f"ob{ci}")
                nc.vector.scalar_tensor_tensor(
                    out=out_sb[:, :cw], in0=pss[ci][:, :cw], scalar=0.0,
                    in1=bias_bc[:, c0 : c0 + cw], op0=Alu.add, op1=Alu.add,
                )
                nc.sync.dma_start(
                    out=out_d[nt * P : (nt + 1) * P, c0 : c0 + cw],
                    in_=out_sb[:, :cw],
                )
    return nc


def make_const_inputs(g=G_FULL):
    return {
        "id128_f32": np.eye(P, dtype=np.float32),
        "idg_f32": np.eye(g, dtype=np.float32),
    }


def kernel(input, qweight, qzeros, scales, bias):
    """Full-problem entry point: shard, run on 8 cores, gather."""
    from concourse.bass_utils import run_bass_kernel_spmd

    nc = bacc.Bacc("TRN2", target_bir_lowering=False, debug=False)
    build(nc)
    nc.compile()

    consts = make_const_inputs()
    x = np.ascontiguousarray(input, dtype=np.float32)
    in_maps = []
    for c in range(N_CORES):
        j0, j1 = c * IC_SHARD, (c + 1) * IC_SHARD
        in_maps.append(
            {
                "qweight": np.ascontiguousarray(qweight[:, j0:j1]),
                "qzeros": np.ascontiguousarray(
                    qzeros[:, c * (IC_SHARD // PACK) : (c + 1) * (IC_SHARD // PACK)]
                ),
                "scales": np.ascontiguousarray(scales[:, j0:j1]),
                "x": x,
                "bias": np.ascontiguousarray(bias[j0:j1]),
                **consts,
            }
        )
    res = run_bass_kernel_spmd(nc, in_maps, list(range(N_CORES)))
    outs = [np.asarray(res.results[c]["out"], dtype=np.float32) for c in range(N_CORES)]
    return np.concatenate(outs, axis=1)
